# revision 1
# baseline (speedup 1.0000x reference)
"""GQA decode attention (B=64, T=1, HQ=16, HK=4, D=128, S_PAST=4095) on 8 TRN2 cores.

Sharding: data-parallel over batch — core i computes batches [8i, 8i+8).
Each core runs the full pipeline on-device: QKV projections, per-head RMSNorm,
RoPE (angles from pos computed on-chip via PE outer-product + Sin activation),
KV-cache append, softmax attention over 4096 positions, output projection.

Host-side prep is layout-only: batch slicing, weights stored transposed
(k-major) so DMA tiles are contiguous, pos cast int64->f32 (values < 4096,
exact), plus constant tables (inv_freq, identity, ones).
"""
import numpy as np
from contextlib import ExitStack

B, DIM = 64, 2048
HQ, HK, D = 16, 4, 128
G = HQ // HK
S_PAST = 4095
EPS = 1e-5
MIN_TS, MAX_TS = 1.0, 10000.0
N_CORES = 8
BL = B // N_CORES  # 8 batches per core
EXP_SHIFT = 30.0   # fixed softmax shift; |score| <= |q||k| ~ 160 worst case,
                   # realistic max ~65 -> exp(s-30) safely inside f32 range

PI = float(np.pi)


def _build_nc(s_past: int):
    import concourse.bass as bass
    import concourse.mybir as mybir
    from concourse import tile
    from concourse.bacc import Bacc

    f32 = mybir.dt.float32
    AF = mybir.ActivationFunctionType
    s_full = s_past + 1
    assert s_full % 128 == 0
    NCH = s_full // 128          # 128-row chunks of kv length
    assert NCH % 8 == 0 or NCH < 8
    CG = 8 if NCH >= 8 else NCH  # chunks per exp group
    half1 = NCH // 2             # first contiguous DMA, in chunks

    nc = Bacc()

    def reg_const(value):
        t = nc.alloc_sbuf_tensor(f"const-f32-{value}", [128, 1], f32)
        nc.gpsimd.memset(t.ap(), value)
        nc.const_aps.aps[(f32, value)] = t.ap()

    for _v in (PI / 2, EPS, 1.0 / D, -EXP_SHIFT):
        reg_const(float(_v))
    nc.all_engine_barrier()

    x_d = nc.declare_dram_parameter("x", [BL, DIM], f32, isOutput=False)
    cos_d = nc.declare_dram_parameter("cos_t", [BL, D // 2], f32, isOutput=False)
    sin_d = nc.declare_dram_parameter("sin_t", [BL, D // 2], f32, isOutput=False)
    kc_d = nc.declare_dram_parameter("k_cache", [BL, HK, s_past, D], f32, isOutput=False)
    vc_d = nc.declare_dram_parameter("v_cache", [BL, HK, s_past, D], f32, isOutput=False)
    wqt_d = nc.declare_dram_parameter("WqT", [DIM, HQ * D], f32, isOutput=False)
    wkt_d = nc.declare_dram_parameter("WkT", [DIM, HK * D], f32, isOutput=False)
    wvt_d = nc.declare_dram_parameter("WvT", [DIM, HK * D], f32, isOutput=False)
    wot_d = nc.declare_dram_parameter("WoT", [HQ * D, DIM], f32, isOutput=False)
    qnw_d = nc.declare_dram_parameter("qn_w", [1, D], f32, isOutput=False)
    knw_d = nc.declare_dram_parameter("kn_w", [1, D], f32, isOutput=False)
    id_d = nc.declare_dram_parameter("ident", [128, 128], f32, isOutput=False)
    onc_d = nc.declare_dram_parameter("ones_col", [128, 1], f32, isOutput=False)
    onr_d = nc.declare_dram_parameter("ones_row", [1, 128], f32, isOutput=False)
    zr_d = nc.declare_dram_parameter("zeros_row", [1, 128], f32, isOutput=False)
    y_d = nc.declare_dram_parameter("y", [BL, DIM], f32, isOutput=True)

    with tile.TileContext(nc) as tc, ExitStack() as ctx:
        cpool = ctx.enter_context(tc.tile_pool(name="const", bufs=1))
        spool = ctx.enter_context(tc.tile_pool(name="small", bufs=1))

        ident = cpool.tile([128, 128], f32)
        nc.gpsimd.dma_start(ident[:], id_d[:])
        ones_col = cpool.tile([128, 1], f32)
        nc.gpsimd.dma_start(ones_col[:], onc_d[:])
        ones_row = cpool.tile([1, 128], f32)
        nc.gpsimd.dma_start(ones_row[:], onr_d[:])
        qnw = cpool.tile([1, D], f32)
        nc.gpsimd.dma_start(qnw[:], qnw_d[:])
        knw = cpool.tile([1, D], f32)
        nc.gpsimd.dma_start(knw[:], knw_d[:])
        sin_t = cpool.tile([BL, D // 2], f32)
        nc.gpsimd.dma_start(sin_t[:], sin_d[:])
        cos_t = cpool.tile([BL, D // 2], f32)
        nc.gpsimd.dma_start(cos_t[:], cos_d[:])
        x_sb = cpool.tile([BL, DIM], f32)
        nc.gpsimd.dma_start(x_sb[:], x_d[:])

        # ---- phase 1: x.T chunks  [128, 16*BL], col kc*BL+b ----
        xT = cpool.tile([128, 16 * BL], f32)
        with tc.tile_pool(name="ph1ps", bufs=2, space="PSUM") as pps:
            # warm PE's clock on ident's DMA so later transposes carry a
            # single sync wait (walrus S3_LW allows only one)
            warm = pps.tile([128, 128], f32, tag="warm")
            nc.tensor.transpose(warm[:], ident[:], ident[:])
            for kc in range(16):
                ps = pps.tile([128, BL], f32)
                nc.tensor.transpose(ps[:], x_sb[:, kc * 128:(kc + 1) * 128],
                                    ident[0:BL, 0:BL])
                nc.vector.tensor_copy(xT[:, kc * BL:(kc + 1) * BL], ps[:])

        # ---- phase 2: projections -> q_bj [BL,2048], k_bj/v_bj [BL,512] ----
        q_bj = cpool.tile([BL, HQ * D], f32)
        k_bj = cpool.tile([BL, HK * D], f32)
        v_bj = cpool.tile([BL, HK * D], f32)
        for wt_d, n_size, dst in ((wqt_d, HQ * D, q_bj), (wkt_d, HK * D, k_bj),
                                  (wvt_d, HK * D, v_bj)):
            nspans = n_size // 512
            with tc.tile_pool(name="wproj", bufs=2) as wpool, \
                 tc.tile_pool(name="projps", bufs=nspans, space="PSUM") as ppool:
                pstiles = [ppool.tile([BL, 512], f32, tag="projps", name=f"projps{_i}") for _i in range(nspans)]
                for kc in range(16):
                    wt = wpool.tile([128, n_size], f32)
                    nc.gpsimd.dma_start(wt[:], wt_d[kc * 128:(kc + 1) * 128, :])
                    for sp in range(nspans):
                        nc.tensor.matmul(pstiles[sp][:], lhsT=xT[:, kc * BL:(kc + 1) * BL],
                                         rhs=wt[:, sp * 512:(sp + 1) * 512],
                                         start=(kc == 0), stop=(kc == 15))
                for sp in range(nspans):
                    nc.vector.tensor_copy(dst[:, sp * 512:(sp + 1) * 512], pstiles[sp][:])

        # ---- phase 3: rope tables + rmsnorm + rope ----
        q_rot = cpool.tile([BL, HQ * D], f32)
        k_rot = cpool.tile([BL, HK * D], f32)
        with tc.tile_pool(name="ph3ps", bufs=4, space="PSUM") as pps, \
             tc.tile_pool(name="ph3sb", bufs=4) as tpool:
            tabs = {}
            for nm, w_sb in (("q", qnw), ("k", knw)):
                wb = pps.tile([BL, D], f32)
                nc.tensor.matmul(wb[:], lhsT=ones_row[:, 0:BL], rhs=w_sb[:],
                                 start=True, stop=True)
                cosA = spool.tile([BL, D // 2], f32, name=f"cosA_{nm}".format(nm=nm))
                nc.vector.tensor_mul(cosA[:], cos_t[:], wb[:, 0:D // 2])
                sinA = spool.tile([BL, D // 2], f32, name=f"sinA_{nm}".format(nm=nm))
                nc.vector.tensor_mul(sinA[:], sin_t[:], wb[:, D // 2:D])
                cosB = spool.tile([BL, D // 2], f32, name=f"cosB_{nm}".format(nm=nm))
                nc.vector.tensor_mul(cosB[:], cos_t[:], wb[:, D // 2:D])
                sinB = spool.tile([BL, D // 2], f32, name=f"sinB_{nm}".format(nm=nm))
                nc.vector.tensor_mul(sinB[:], sin_t[:], wb[:, 0:D // 2])
                tabs[nm] = (cosA, sinA, cosB, sinB)

            for src, dst, nheads, nm in ((q_bj, q_rot, HQ, "q"), (k_bj, k_rot, HK, "k")):
                cosA, sinA, cosB, sinB = tabs[nm]
                for hh in range(nheads):
                    base = hh * D
                    hsl = src[:, base:base + D]
                    sq = tpool.tile([BL, D], f32, tag="sq")
                    nc.vector.tensor_mul(sq[:], hsl, hsl)
                    ssum = tpool.tile([BL, 1], f32, tag="ssum")
                    nc.vector.tensor_reduce(ssum[:], sq[:], axis=mybir.AxisListType.X,
                                            op=mybir.AluOpType.add)
                    sstd = tpool.tile([BL, 1], f32, tag="sstd")
                    nc.scalar.activation(sstd[:], ssum[:], AF.Sqrt, scale=1.0 / D, bias=EPS)
                    rstd = tpool.tile([BL, 1], f32, tag="rstd")
                    nc.vector.reciprocal(rstd[:], sstd[:])
                    qn = tpool.tile([BL, D], f32, tag="qn")
                    nc.vector.tensor_scalar_mul(qn[:], hsl, rstd[:])
                    h1, h2 = qn[:, 0:D // 2], qn[:, D // 2:D]
                    t1 = tpool.tile([BL, D // 2], f32, tag="t1")
                    nc.vector.tensor_mul(t1[:], h1, cosA[:])
                    t2 = tpool.tile([BL, D // 2], f32, tag="t2")
                    nc.vector.tensor_mul(t2[:], h2, sinA[:])
                    nc.vector.tensor_sub(dst[:, base:base + D // 2], t1[:], t2[:])
                    t3 = tpool.tile([BL, D // 2], f32, tag="t3")
                    nc.vector.tensor_mul(t3[:], h2, cosB[:])
                    t4 = tpool.tile([BL, D // 2], f32, tag="t4")
                    nc.vector.tensor_mul(t4[:], h1, sinB[:])
                    nc.vector.tensor_add(dst[:, base + D // 2:base + D], t3[:], t4[:])

        # ---- phase 4: q.T  [128, 16*BL], col qh*BL+b ; k_new.T [128, 4*BL] ----
        qT = cpool.tile([128, HQ * BL], f32)
        kTn = cpool.tile([128, HK * BL], f32)
        with tc.tile_pool(name="ph4ps", bufs=2, space="PSUM") as pps:
            for qh in range(HQ):
                ps = pps.tile([128, BL], f32)
                nc.tensor.transpose(ps[:], q_rot[:, qh * D:(qh + 1) * D],
                                    ident[0:BL, 0:BL])
                nc.vector.tensor_copy(qT[:, qh * BL:(qh + 1) * BL], ps[:])
            for hh in range(HK):
                ps = pps.tile([128, BL], f32)
                nc.tensor.transpose(ps[:], k_rot[:, hh * D:(hh + 1) * D],
                                    ident[0:BL, 0:BL])
                nc.vector.tensor_copy(kTn[:, hh * BL:(hh + 1) * BL], ps[:])

        # ---- phase 5: attention ----
        # attn cols: (b*HK+h)*G+g = b*16 + qh ;  denominator same layout
        attn_n = cpool.tile([128, BL * HQ], f32)
        recip = spool.tile([1, BL * HQ], f32)
        with tc.tile_pool(name="kv", bufs=4) as kvpool, \
             tc.tile_pool(name="kts", bufs=4) as ktspool, \
             tc.tile_pool(name="exps", bufs=3) as expool, \
             tc.tile_pool(name="attnps", bufs=1, space="PSUM") as apspool, \
             tc.tile_pool(name="ktps", bufs=2, space="PSUM") as ktppool, \
             tc.tile_pool(name="scps", bufs=2, space="PSUM") as scppool:
            attn_ps = apspool.tile([128, BL * HQ], f32, tag="attn")
            den_ps = apspool.tile([1, BL * HQ], f32, tag="den")
            for b in range(BL):
                for h in range(HK):
                    bh = b * HK + h
                    kt = kvpool.tile([128, s_full], f32, tag="k")
                    vt = kvpool.tile([128, s_full], f32, tag="v")
                    for tl, cache in ((kt, kc_d), (vt, vc_d)):
                        nc.gpsimd.dma_start(
                            tl[:, 0:half1 * 128].rearrange("p (c d) -> p c d", d=128),
                            cache[b, h, 0:half1 * 128, :].rearrange(
                                "(c p) d -> p c d", p=128))
                        if NCH - 1 > half1:
                            nc.gpsimd.dma_start(
                                tl[:, half1 * 128:(NCH - 1) * 128].rearrange(
                                    "p (c d) -> p c d", d=128),
                                cache[b, h, half1 * 128:(NCH - 1) * 128, :].rearrange(
                                    "(c p) d -> p c d", p=128))
                        nc.gpsimd.dma_start(
                            tl[0:127, (NCH - 1) * 128:s_full],
                            cache[b, h, (NCH - 1) * 128:s_past, :])
                        nc.gpsimd.dma_start(
                            tl[127:128, (NCH - 1) * 128:s_full], zr_d[:])
                    # partition 127 of the last chunk is garbage; every consumer
                    # below restricts that chunk's contraction to 127 rows. The
                    # appended (s=4095) position is handled via k_rot/v_bj
                    # directly in tiny rank-1 matmuls at the end of the group.
                    rq = qT[:, h * G * BL + b: h * G * BL + b + (G - 1) * BL + 1: BL]
                    for cg in range(NCH // CG):
                        sc = scppool.tile([128, CG * G], f32)
                        for j in range(CG):
                            c = cg * CG + j
                            ktp = ktppool.tile([128, 128], f32)
                            nc.tensor.transpose(ktp[:], kt[:, c * 128:(c + 1) * 128],
                                                ident[:])
                            kts = ktspool.tile([128, 128], f32)
                            nc.vector.tensor_copy(kts[:], ktp[:])
                            ns = 127 if c == NCH - 1 else 128
                            nc.tensor.matmul(sc[0:ns, j * G:(j + 1) * G],
                                             lhsT=kts[:, 0:ns],
                                             rhs=rq, start=True, stop=True)
                        ex = expool.tile([128, CG * G], f32)
                        if cg == NCH // CG - 1:
                            nc.scalar.activation(ex[:, 0:(CG - 1) * G],
                                                 sc[:, 0:(CG - 1) * G],
                                                 AF.Exp, bias=-EXP_SHIFT)
                            nc.scalar.activation(ex[0:127, (CG - 1) * G:CG * G],
                                                 sc[0:127, (CG - 1) * G:CG * G],
                                                 AF.Exp, bias=-EXP_SHIFT)
                        else:
                            nc.scalar.activation(ex[:], sc[:], AF.Exp, bias=-EXP_SHIFT)
                        for j in range(CG):
                            c = cg * CG + j
                            ns = 127 if c == NCH - 1 else 128
                            nc.tensor.matmul(attn_ps[:, bh * G:(bh + 1) * G],
                                             lhsT=vt[0:ns, c * 128:(c + 1) * 128],
                                             rhs=ex[0:ns, j * G:(j + 1) * G],
                                             start=(c == 0), stop=False,
                                             skip_group_check=True)
                            nc.tensor.matmul(den_ps[:, bh * G:(bh + 1) * G],
                                             lhsT=ones_col[0:ns],
                                             rhs=ex[0:ns, j * G:(j + 1) * G],
                                             start=(c == 0), stop=False,
                                             skip_group_check=True)
                    # appended position s = s_past (new token)
                    scn = scppool.tile([1, G], f32, tag="scn", bufs=1)
                    nc.tensor.matmul(scn[:], lhsT=kTn[:, h * BL + b:h * BL + b + 1],
                                     rhs=rq, start=True, stop=True)
                    exn = expool.tile([1, G], f32, tag="exn")
                    nc.scalar.activation(exn[:], scn[:], AF.Exp, bias=-EXP_SHIFT)
                    vnew = expool.tile([1, D], f32, tag="vnew")
                    nc.gpsimd.dma_start(vnew[:], v_bj[b:b + 1, h * D:(h + 1) * D])
                    nc.tensor.matmul(attn_ps[:, bh * G:(bh + 1) * G],
                                     lhsT=vnew[:],
                                     rhs=exn[:], start=False, stop=True,
                                     skip_group_check=True)
                    nc.tensor.matmul(den_ps[:, bh * G:(bh + 1) * G],
                                     lhsT=ones_col[0:1],
                                     rhs=exn[:], start=False, stop=True,
                                     skip_group_check=True)
            # normalize: attn_n = attn_ps * (1/den) broadcast down partitions
            nc.vector.reciprocal(recip[:], den_ps[:])
            with tc.tile_pool(name="bcps", bufs=1, space="PSUM") as bcpool:
                bc = bcpool.tile([128, BL * HQ], f32)
                nc.tensor.matmul(bc[:], lhsT=ones_row[:], rhs=recip[:],
                                 start=True, stop=True)
                bcs = spool.tile([128, BL * HQ], f32)
                nc.vector.tensor_copy(bcs[:], bc[:])
                nc.vector.tensor_mul(attn_n[:], attn_ps[:], bcs[:])

        # ---- phase 6: o_proj  y.T[n,b] = sum_j WoT[j,n] attnT[j,b] ----
        y_sb = cpool.tile([BL, DIM], f32)
        for half in range(2):
            n0 = half * (DIM // 2)
            with tc.tile_pool(name="wo", bufs=1) as wopool, \
                 tc.tile_pool(name="yps", bufs=2, space="PSUM") as ypool, \
                 tc.tile_pool(name="ysb", bufs=4) as ysbpool, \
                 tc.tile_pool(name="ytps", bufs=2, space="PSUM") as ytpool:
                wts = []
                for qh in range(HQ):
                    wt = wopool.tile([128, DIM // 2], f32, tag="wo",
                                     name=f"wo{half}_{qh}", bufs=HQ)
                    nc.gpsimd.dma_start(wt[:], wot_d[qh * 128:(qh + 1) * 128,
                                                   n0:n0 + DIM // 2])
                    wts.append(wt)
                for nh in range(8):
                    yps = ypool.tile([128, BL], f32, tag="yps")
                    for qh in range(HQ):
                        rhs_a = attn_n[:, qh: qh + (BL - 1) * HQ + 1: HQ]
                        nc.tensor.matmul(yps[:],
                                         lhsT=wts[qh][:, nh * 128:(nh + 1) * 128],
                                         rhs=rhs_a, start=(qh == 0), stop=(qh == HQ - 1))
                    ys = ysbpool.tile([128, BL], f32, tag="ysb")
                    nc.vector.tensor_copy(ys[:], yps[:])
                    yt = ytpool.tile([BL, 128], f32, tag="yt")
                    nc.tensor.transpose(yt[:], ys[:], ident[0:128, 0:128])
                    nc.vector.tensor_copy(
                        y_sb[:, n0 + nh * 128:n0 + (nh + 1) * 128], yt[:])
        nc.gpsimd.dma_start(y_d[:], y_sb[:])

    return nc


_NC_CACHE = {}


def _get_nc(s_past: int):
    if s_past not in _NC_CACHE:
        nc = _build_nc(s_past)
        if not nc.is_finalized():
            nc.finalize()
        _NC_CACHE[s_past] = nc
    return _NC_CACHE[s_past]


def _host_prep(x, pos, k_cache, v_cache, Wq, Wk, Wv, Wo, qn_w, kn_w, s_past):
    half = D // 2
    fraction = 2.0 * np.arange(half, dtype=np.float64) / D
    inv_freq = 1.0 / (MIN_TS * (MAX_TS / MIN_TS) ** fraction)
    shared = {
        "WqT": np.ascontiguousarray(np.asarray(Wq, np.float32).T),
        "WkT": np.ascontiguousarray(np.asarray(Wk, np.float32).T),
        "WvT": np.ascontiguousarray(np.asarray(Wv, np.float32).T),
        "WoT": np.ascontiguousarray(np.asarray(Wo, np.float32).T),
        "qn_w": np.asarray(qn_w, np.float32).reshape(1, D),
        "kn_w": np.asarray(kn_w, np.float32).reshape(1, D),
        "ident": np.eye(128, dtype=np.float32),
        "ones_col": np.ones((128, 1), np.float32),
        "ones_row": np.ones((1, 128), np.float32),
        "zeros_row": np.zeros((1, 128), np.float32),
    }
    x = np.asarray(x, np.float32).reshape(B, DIM)
    pos_f = np.asarray(pos, np.float64).reshape(B)
    freqs = pos_f[:, None] * inv_freq[None, :]
    cos_t = np.cos(freqs).astype(np.float32)
    sin_t = np.sin(freqs).astype(np.float32)
    k_cache = np.asarray(k_cache, np.float32)
    v_cache = np.asarray(v_cache, np.float32)
    in_maps = []
    for i in range(N_CORES):
        sl = slice(i * BL, (i + 1) * BL)
        m = dict(shared)
        m["x"] = np.ascontiguousarray(x[sl])
        m["cos_t"] = np.ascontiguousarray(cos_t[sl])
        m["sin_t"] = np.ascontiguousarray(sin_t[sl])
        m["k_cache"] = np.ascontiguousarray(k_cache[sl])
        m["v_cache"] = np.ascontiguousarray(v_cache[sl])
        in_maps.append(m)
    return in_maps


def kernel(x, pos, k_cache, v_cache, Wq, Wk, Wv, Wo, qn_w, kn_w):
    from concourse.bass_utils import run_bass_kernel_spmd
    nc = _get_nc(S_PAST)
    in_maps = _host_prep(x, pos, k_cache, v_cache, Wq, Wk, Wv, Wo, qn_w, kn_w, S_PAST)
    res = run_bass_kernel_spmd(nc, in_maps, list(range(N_CORES)))
    y = np.concatenate([res.results[i]["y"].reshape(BL, 1, DIM)
                        for i in range(N_CORES)], axis=0)
    return y



# revision 6
# speedup vs baseline: 2.6077x; 2.6077x over previous
"""GQA decode attention (B=64, T=1, HQ=16, HK=4, D=128, S_PAST=4095) on 8 TRN2 cores.

Under axon, wall time is dominated by the host->device tunnel (~60 MB/s), so the
design minimizes uploaded bytes and per-call host overhead:

  * Sharding: core c = (kv-head h = c%4, batch-half g = c//4). Each core gets
    its KV slice exactly once (no duplication) and only the weight slices for
    its 4 q-heads / 1 kv-head (~5 MB vs 42 MB full copy). Each core returns a
    partial y^T (its heads' o_proj contribution); host sums 4 partials per
    batch half.
  * dtypes over the wire: K cache f16 (upcast to f32 on device for the score
    matmuls), V cache bf16 (matches bf16 softmax weights in the PV matmul --
    bf16 is required there because exp(score-30) can exceed f16 max), weights
    f16, x f16. Total upload ~585 MB vs 1410 MB for the f32 data-parallel
    version.
  * Runner: the PJRT executable is built once and cached; global (stacked)
    input arrays are prebuilt in host prep, so a device call is transfer +
    execute + download only (no per-call np.concatenate / retrace).

Device pipeline per core: QKV projections (f16 matmul, f32 psum), per-head
RMSNorm + RoPE in f32, softmax attention over 4096 positions with a fixed
exp shift, o_proj for the core's 4 q-heads.
"""
import numpy as np
from contextlib import ExitStack

B, DIM = 64, 2048
HQ, HK, D = 16, 4, 128
G = HQ // HK          # 4 q-heads per kv head
S_PAST = 4095
S_FULL = S_PAST + 1
EPS = 1e-5
MIN_TS, MAX_TS = 1.0, 10000.0
N_CORES = 8
NB = 2                # batch halves
BL = B // NB          # 32 batches per core
NCH = S_FULL // 128   # 32 kv chunks of 128 positions
CG = 8                # chunks per exp group
EXP_SHIFT = 30.0      # fixed softmax shift; scores stay < ~60 so bf16/f32 hold


def _build_nc():
    import concourse.mybir as mybir
    from concourse import tile
    from concourse.bacc import Bacc

    f32 = mybir.dt.float32
    f16 = mybir.dt.float16
    bf16 = mybir.dt.bfloat16
    AF = mybir.ActivationFunctionType
    half1 = NCH // 2

    nc = Bacc()

    def reg_const(value):
        t = nc.alloc_sbuf_tensor(f"const-f32-{value}", [128, 1], f32)
        nc.gpsimd.memset(t.ap(), value)
        nc.const_aps.aps[(f32, value)] = t.ap()

    for _v in (EPS, 1.0 / D, -EXP_SHIFT):
        reg_const(float(_v))
    nc.all_engine_barrier()

    x_d = nc.declare_dram_parameter("x", [BL, DIM], f16, isOutput=False)
    cos_d = nc.declare_dram_parameter("cos_t", [BL, D // 2], f32, isOutput=False)
    sin_d = nc.declare_dram_parameter("sin_t", [BL, D // 2], f32, isOutput=False)
    kc_d = nc.declare_dram_parameter("k_cache", [BL, S_PAST, D], f16, isOutput=False)
    vc_d = nc.declare_dram_parameter("v_cache", [BL, S_PAST, D], bf16, isOutput=False)
    wq_d = nc.declare_dram_parameter("WqT", [DIM, G * D], f16, isOutput=False)
    wkv_d = nc.declare_dram_parameter("WkvT", [DIM, 2 * D], f16, isOutput=False)
    wo_d = nc.declare_dram_parameter("WoT", [G * D, DIM], f16, isOutput=False)
    qnw_d = nc.declare_dram_parameter("qn_w", [1, D], f32, isOutput=False)
    knw_d = nc.declare_dram_parameter("kn_w", [1, D], f32, isOutput=False)
    id32_d = nc.declare_dram_parameter("id32", [128, 128], f32, isOutput=False)
    id16_d = nc.declare_dram_parameter("id16", [128, 128], f16, isOutput=False)
    yt_d = nc.declare_dram_parameter("yT", [DIM, BL], f32, isOutput=True)

    with tile.TileContext(nc) as tc, ExitStack() as ctx:
        cpool = ctx.enter_context(tc.tile_pool(name="const", bufs=1))
        spool = ctx.enter_context(tc.tile_pool(name="small", bufs=1))

        id32 = cpool.tile([128, 128], f32)
        nc.gpsimd.dma_start(id32[:], id32_d[:])
        id16 = cpool.tile([128, 128], f16)
        nc.gpsimd.dma_start(id16[:], id16_d[:])
        ones_row = cpool.tile([1, 128], f32)
        nc.gpsimd.memset(ones_row[:], 1.0)
        ones_col = cpool.tile([128, 1], bf16)
        nc.gpsimd.memset(ones_col[:], 1.0)
        zrow16 = cpool.tile([1, 128], f16)
        nc.gpsimd.memset(zrow16[:], 0.0)
        zrowb = cpool.tile([1, 128], bf16)
        nc.gpsimd.memset(zrowb[:], 0.0)
        qnw = cpool.tile([1, D], f32)
        nc.gpsimd.dma_start(qnw[:], qnw_d[:])
        knw = cpool.tile([1, D], f32)
        nc.gpsimd.dma_start(knw[:], knw_d[:])
        sin_t = cpool.tile([BL, D // 2], f32)
        nc.gpsimd.dma_start(sin_t[:], sin_d[:])
        cos_t = cpool.tile([BL, D // 2], f32)
        nc.gpsimd.dma_start(cos_t[:], cos_d[:])
        x_sb = cpool.tile([BL, DIM], f16)
        nc.gpsimd.dma_start(x_sb[:], x_d[:])

        # ---- phase 1: x.T chunks  [128, 16*BL], col kc*BL+b (f16) ----
        xT = cpool.tile([128, 16 * BL], f16)
        with tc.tile_pool(name="ph1ps", bufs=2, space="PSUM") as pps:
            # warm PE's clock on the ident DMA so later transposes carry a
            # single sync wait (walrus S3_LW allows only one)
            warm = pps.tile([128, 128], f16, tag="warm")
            nc.tensor.transpose(warm[:], id16[:], id16[:])
            for kc in range(16):
                ps = pps.tile([128, BL], f16)
                nc.tensor.transpose(ps[:], x_sb[:, kc * 128:(kc + 1) * 128],
                                    id16[0:BL, 0:BL])
                nc.vector.tensor_copy(xT[:, kc * BL:(kc + 1) * BL], ps[:])

        # ---- phase 2: projections -> q_bj [BL,512] f32, k/v_bj [BL,128] ----
        q_bj = cpool.tile([BL, G * D], f32)
        kv_bj = cpool.tile([BL, 2 * D], f32)
        with tc.tile_pool(name="wproj", bufs=2) as wpool, \
             tc.tile_pool(name="projps", bufs=2, space="PSUM") as ppool:
            pq = ppool.tile([BL, G * D], f32, tag="pq")
            pkv = ppool.tile([BL, 2 * D], f32, tag="pkv")
            for kc in range(16):
                wqt = wpool.tile([128, G * D], f16, tag="wq")
                nc.gpsimd.dma_start(wqt[:], wq_d[kc * 128:(kc + 1) * 128, :])
                wkvt = wpool.tile([128, 2 * D], f16, tag="wkv")
                nc.gpsimd.dma_start(wkvt[:], wkv_d[kc * 128:(kc + 1) * 128, :])
                lt = xT[:, kc * BL:(kc + 1) * BL]
                nc.tensor.matmul(pq[:], lhsT=lt, rhs=wqt[:],
                                 start=(kc == 0), stop=(kc == 15))
                nc.tensor.matmul(pkv[:], lhsT=lt, rhs=wkvt[:],
                                 start=(kc == 0), stop=(kc == 15))
            nc.vector.tensor_copy(q_bj[:], pq[:])
            nc.vector.tensor_copy(kv_bj[:], pkv[:])

        # ---- phase 3: rope tables + rmsnorm + rope (f32) ----
        q_rot = cpool.tile([BL, G * D], f32)
        k_rot = cpool.tile([BL, D], f32)
        with tc.tile_pool(name="ph3ps", bufs=4, space="PSUM") as pps, \
             tc.tile_pool(name="ph3sb", bufs=4) as tpool:
            tabs = {}
            for nm, w_sb in (("q", qnw), ("k", knw)):
                wb = pps.tile([BL, D], f32)
                nc.tensor.matmul(wb[:], lhsT=ones_row[:, 0:BL], rhs=w_sb[:],
                                 start=True, stop=True)
                cosA = spool.tile([BL, D // 2], f32, name=f"cosA_{nm}")
                nc.vector.tensor_mul(cosA[:], cos_t[:], wb[:, 0:D // 2])
                sinA = spool.tile([BL, D // 2], f32, name=f"sinA_{nm}")
                nc.vector.tensor_mul(sinA[:], sin_t[:], wb[:, D // 2:D])
                cosB = spool.tile([BL, D // 2], f32, name=f"cosB_{nm}")
                nc.vector.tensor_mul(cosB[:], cos_t[:], wb[:, D // 2:D])
                sinB = spool.tile([BL, D // 2], f32, name=f"sinB_{nm}")
                nc.vector.tensor_mul(sinB[:], sin_t[:], wb[:, 0:D // 2])
                tabs[nm] = (cosA, sinA, cosB, sinB)

            for src, dst, nheads, nm in ((q_bj, q_rot, G, "q"),
                                         (kv_bj, k_rot, 1, "k")):
                cosA, sinA, cosB, sinB = tabs[nm]
                for hh in range(nheads):
                    base = hh * D
                    hsl = src[:, base:base + D]
                    sq = tpool.tile([BL, D], f32, tag="sq")
                    nc.vector.tensor_mul(sq[:], hsl, hsl)
                    ssum = tpool.tile([BL, 1], f32, tag="ssum")
                    nc.vector.tensor_reduce(ssum[:], sq[:], axis=mybir.AxisListType.X,
                                            op=mybir.AluOpType.add)
                    sstd = tpool.tile([BL, 1], f32, tag="sstd")
                    nc.scalar.activation(sstd[:], ssum[:], AF.Sqrt, scale=1.0 / D, bias=EPS)
                    rstd = tpool.tile([BL, 1], f32, tag="rstd")
                    nc.vector.reciprocal(rstd[:], sstd[:])
                    qn = tpool.tile([BL, D], f32, tag="qn")
                    nc.vector.tensor_scalar_mul(qn[:], hsl, rstd[:])
                    h1, h2 = qn[:, 0:D // 2], qn[:, D // 2:D]
                    t1 = tpool.tile([BL, D // 2], f32, tag="t1")
                    nc.vector.tensor_mul(t1[:], h1, cosA[:])
                    t2 = tpool.tile([BL, D // 2], f32, tag="t2")
                    nc.vector.tensor_mul(t2[:], h2, sinA[:])
                    nc.vector.tensor_sub(dst[:, base:base + D // 2], t1[:], t2[:])
                    t3 = tpool.tile([BL, D // 2], f32, tag="t3")
                    nc.vector.tensor_mul(t3[:], h2, cosB[:])
                    t4 = tpool.tile([BL, D // 2], f32, tag="t4")
                    nc.vector.tensor_mul(t4[:], h1, sinB[:])
                    nc.vector.tensor_add(dst[:, base + D // 2:base + D], t3[:], t4[:])

        # ---- phase 4: q.T [128, G*BL] f32 ; k_new.T [128, BL] f32 ;
        #               v_bj as bf16 for the new-token PV matmul ----
        qT = cpool.tile([128, G * BL], f32)
        kTn = cpool.tile([128, BL], f32)
        v_bj16 = cpool.tile([BL, D], bf16)
        nc.vector.tensor_copy(v_bj16[:], kv_bj[:, D:2 * D])
        with tc.tile_pool(name="ph4ps", bufs=2, space="PSUM") as pps:
            for j in range(G):
                ps = pps.tile([128, BL], f32)
                nc.tensor.transpose(ps[:], q_rot[:, j * D:(j + 1) * D],
                                    id32[0:BL, 0:BL])
                nc.vector.tensor_copy(qT[:, j * BL:(j + 1) * BL], ps[:])
            ps = pps.tile([128, BL], f32)
            nc.tensor.transpose(ps[:], k_rot[:], id32[0:BL, 0:BL])
            nc.vector.tensor_copy(kTn[:], ps[:])

        # ---- phase 5: attention ----
        # psum cols: b*G + j  (j = local q-head)
        attn_n = cpool.tile([128, BL * G], f16)
        recip = spool.tile([1, BL * G], f32)
        with tc.tile_pool(name="kv", bufs=4) as kvpool, \
             tc.tile_pool(name="kts", bufs=4) as ktspool, \
             tc.tile_pool(name="exps", bufs=3) as expool, \
             tc.tile_pool(name="attnps", bufs=1, space="PSUM") as apspool, \
             tc.tile_pool(name="ktps", bufs=2, space="PSUM") as ktppool, \
             tc.tile_pool(name="scps", bufs=2, space="PSUM") as scppool:
            attn_ps = apspool.tile([128, BL * G], f32, tag="attn")
            den_ps = apspool.tile([1, BL * G], f32, tag="den")
            for b in range(BL):
                kt = kvpool.tile([128, S_FULL], f16, tag="k")
                vt = kvpool.tile([128, S_FULL], bf16, tag="v")
                for tl, cache, zr in ((kt, kc_d, zrow16), (vt, vc_d, zrowb)):
                    nc.gpsimd.dma_start(
                        tl[:, 0:half1 * 128].rearrange("p (c d) -> p c d", d=128),
                        cache[b, 0:half1 * 128, :].rearrange(
                            "(c p) d -> p c d", p=128))
                    nc.gpsimd.dma_start(
                        tl[:, half1 * 128:(NCH - 1) * 128].rearrange(
                            "p (c d) -> p c d", d=128),
                        cache[b, half1 * 128:(NCH - 1) * 128, :].rearrange(
                            "(c p) d -> p c d", p=128))
                    nc.gpsimd.dma_start(
                        tl[0:127, (NCH - 1) * 128:S_FULL],
                        cache[b, (NCH - 1) * 128:S_PAST, :])
                    nc.gpsimd.dma_start(tl[127:128, (NCH - 1) * 128:S_FULL], zr[:])
                # partition 127 of the last chunk is garbage; consumers below
                # restrict that chunk to 127 rows. The appended (s=4095)
                # position is handled via kTn/v_bj16 rank-1 matmuls at the end.
                rq = qT[:, b: b + (G - 1) * BL + 1: BL]
                for cg in range(NCH // CG):
                    sc = scppool.tile([128, CG * G], f32)
                    for j in range(CG):
                        c = cg * CG + j
                        ktp = ktppool.tile([128, 128], f16)
                        nc.tensor.transpose(ktp[:], kt[:, c * 128:(c + 1) * 128],
                                            id16[:])
                        kts = ktspool.tile([128, 128], f32)
                        nc.vector.tensor_copy(kts[:], ktp[:])
                        ns = 127 if c == NCH - 1 else 128
                        nc.tensor.matmul(sc[0:ns, j * G:(j + 1) * G],
                                         lhsT=kts[:, 0:ns],
                                         rhs=rq, start=True, stop=True)
                    ex = expool.tile([128, CG * G], bf16)
                    if cg == NCH // CG - 1:
                        nc.scalar.activation(ex[:, 0:(CG - 1) * G],
                                             sc[:, 0:(CG - 1) * G],
                                             AF.Exp, bias=-EXP_SHIFT)
                        nc.scalar.activation(ex[0:127, (CG - 1) * G:CG * G],
                                             sc[0:127, (CG - 1) * G:CG * G],
                                             AF.Exp, bias=-EXP_SHIFT)
                    else:
                        nc.scalar.activation(ex[:], sc[:], AF.Exp, bias=-EXP_SHIFT)
                    for j in range(CG):
                        c = cg * CG + j
                        ns = 127 if c == NCH - 1 else 128
                        nc.tensor.matmul(attn_ps[:, b * G:(b + 1) * G],
                                         lhsT=vt[0:ns, c * 128:(c + 1) * 128],
                                         rhs=ex[0:ns, j * G:(j + 1) * G],
                                         start=(c == 0), stop=False,
                                         skip_group_check=True)
                        nc.tensor.matmul(den_ps[:, b * G:(b + 1) * G],
                                         lhsT=ones_col[0:ns],
                                         rhs=ex[0:ns, j * G:(j + 1) * G],
                                         start=(c == 0), stop=False,
                                         skip_group_check=True)
                # appended position s = S_PAST (new token)
                scn = scppool.tile([1, G], f32, tag="scn", bufs=1)
                nc.tensor.matmul(scn[:], lhsT=kTn[:, b:b + 1],
                                 rhs=rq, start=True, stop=True)
                exn = expool.tile([1, G], bf16, tag="exn")
                nc.scalar.activation(exn[:], scn[:], AF.Exp, bias=-EXP_SHIFT)
                vnew = expool.tile([1, D], bf16, tag="vnew")
                nc.gpsimd.dma_start(vnew[:], v_bj16[b:b + 1, :])
                nc.tensor.matmul(attn_ps[:, b * G:(b + 1) * G],
                                 lhsT=vnew[:],
                                 rhs=exn[:], start=False, stop=True,
                                 skip_group_check=True)
                nc.tensor.matmul(den_ps[:, b * G:(b + 1) * G],
                                 lhsT=ones_col[0:1],
                                 rhs=exn[:], start=False, stop=True,
                                 skip_group_check=True)
            # normalize: attn_n = attn_ps * (1/den) broadcast down partitions
            nc.vector.reciprocal(recip[:], den_ps[:])
            with tc.tile_pool(name="bcps", bufs=1, space="PSUM") as bcpool:
                bc = bcpool.tile([128, BL * G], f32)
                nc.tensor.matmul(bc[:], lhsT=ones_row[:], rhs=recip[:],
                                 start=True, stop=True)
                bcs = spool.tile([128, BL * G], f32)
                nc.vector.tensor_copy(bcs[:], bc[:])
                nc.vector.tensor_mul(attn_n[:], attn_ps[:], bcs[:])

        # ---- phase 6: o_proj  yT[n,b] = sum_{j,d} WoT[(j,d),n] attnT[(j,d),b] ----
        ysT = cpool.tile([128, 16 * BL], f32)
        with tc.tile_pool(name="wo", bufs=1) as wopool, \
             tc.tile_pool(name="yps", bufs=2, space="PSUM") as ypool:
            wts = []
            for j in range(G):
                wt = wopool.tile([128, DIM], f16, tag="wo", name=f"wo{j}", bufs=G)
                nc.gpsimd.dma_start(wt[:], wo_d[j * 128:(j + 1) * 128, :])
                wts.append(wt)
            for nh in range(16):
                yps = ypool.tile([128, BL], f32, tag="yps")
                for j in range(G):
                    rhs_a = attn_n[:, j: j + (BL - 1) * G + 1: G]
                    nc.tensor.matmul(yps[:],
                                     lhsT=wts[j][:, nh * 128:(nh + 1) * 128],
                                     rhs=rhs_a, start=(j == 0), stop=(j == G - 1))
                nc.vector.tensor_copy(ysT[:, nh * BL:(nh + 1) * BL], yps[:])
        nc.gpsimd.dma_start(
            yt_d[:].rearrange("(n p) b -> p n b", p=128),
            ysT[:].rearrange("p (n b) -> p n b", b=BL))

    return nc


_NC = None


def _get_nc():
    global _NC
    if _NC is None:
        nc = _build_nc()
        if not nc.is_finalized():
            nc.finalize()
        _NC = nc
    return _NC


_RUNNER = None


def _get_runner():
    """Build (once) the jitted shard_map executable for the bass module."""
    global _RUNNER
    if _RUNNER is None:
        import jax
        import concourse.mybir as mybir
        from concourse import bass2jax
        from jax.experimental.shard_map import shard_map
        from jax.sharding import Mesh, PartitionSpec

        nc = _get_nc()
        bass2jax.install_neuronx_cc_hook()
        assert nc.dbg_addr is None
        partition_name = (nc.partition_id_tensor.name
                          if nc.partition_id_tensor else None)

        in_names, out_names, out_avals, zero_shapes = [], [], [], []
        for alloc in nc.m.functions[0].allocations:
            if not isinstance(alloc, mybir.MemoryLocationSet):
                continue
            name = alloc.memorylocations[0].name
            if alloc.kind == "ExternalInput":
                if name != partition_name:
                    in_names.append(name)
            elif alloc.kind == "ExternalOutput":
                out_names.append(name)
                shape = tuple(alloc.tensor_shape)
                dtype = mybir.dt.np(alloc.dtype)
                out_avals.append(jax.core.ShapedArray(shape, dtype))
                zero_shapes.append((shape, dtype))
        n_params = len(in_names)
        all_names = list(in_names) + out_names
        if partition_name is not None:
            all_names.append(partition_name)

        def _body(*args):
            operands = list(args)
            if partition_name is not None:
                operands.append(bass2jax.partition_id_tensor())
            outs = bass2jax._bass_exec_p.bind(
                *operands,
                out_avals=tuple(out_avals),
                in_names=tuple(all_names),
                out_names=tuple(out_names),
                lowering_input_output_aliases=(),
                sim_require_finite=True,
                sim_require_nnan=True,
                nc=nc,
            )
            return tuple(outs)

        devices = jax.devices()[:N_CORES]
        mesh = Mesh(np.asarray(devices), ("core",))
        n_outs = len(out_names)
        sharded = jax.jit(
            shard_map(
                _body, mesh=mesh,
                in_specs=(PartitionSpec("core"),) * (n_params + n_outs),
                out_specs=(PartitionSpec("core"),) * n_outs,
                check_rep=False,
            ),
            donate_argnums=tuple(range(n_params, n_params + n_outs)),
            keep_unused=True,
        )
        _RUNNER = (sharded, in_names, zero_shapes)
    return _RUNNER


def _f16(a):
    return np.asarray(a, np.float32).astype(np.float16)


def _bf16(a32):
    """Round-to-nearest-even f32 -> bf16 via integer ops (fast, vectorized)."""
    import ml_dtypes
    u = np.ascontiguousarray(a32, np.float32).view(np.uint32)
    r = ((u + 0x7FFF + ((u >> 16) & 1)) >> 16).astype(np.uint16)
    return r.view(ml_dtypes.bfloat16)


def _host_prep(x, pos, k_cache, v_cache, Wq, Wk, Wv, Wo, qn_w, kn_w):
    """Build the global (core-stacked) input arrays the runner consumes."""
    from concurrent.futures import ThreadPoolExecutor

    x = np.asarray(x, np.float32).reshape(B, DIM)
    pos_f = np.asarray(pos, np.float64).reshape(B)
    half = D // 2
    fraction = 2.0 * np.arange(half, dtype=np.float64) / D
    inv_freq = 1.0 / (MIN_TS * (MAX_TS / MIN_TS) ** fraction)
    freqs = pos_f[:, None] * inv_freq[None, :]
    cos_t = np.cos(freqs).astype(np.float32)
    sin_t = np.sin(freqs).astype(np.float32)
    k_cache = np.asarray(k_cache, np.float32)
    v_cache = np.asarray(v_cache, np.float32)
    Wq = np.asarray(Wq, np.float32)
    Wk = np.asarray(Wk, np.float32)
    Wv = np.asarray(Wv, np.float32)
    Wo = np.asarray(Wo, np.float32)

    import ml_dtypes
    g = {
        "x": np.empty((N_CORES * BL, DIM), np.float16),
        "cos_t": np.empty((N_CORES * BL, half), np.float32),
        "sin_t": np.empty((N_CORES * BL, half), np.float32),
        "k_cache": np.empty((N_CORES * BL, S_PAST, D), np.float16),
        "v_cache": np.empty((N_CORES * BL, S_PAST, D), ml_dtypes.bfloat16),
        "WqT": np.empty((N_CORES * DIM, G * D), np.float16),
        "WkvT": np.empty((N_CORES * DIM, 2 * D), np.float16),
        "WoT": np.empty((N_CORES * G * D, DIM), np.float16),
        "qn_w": np.tile(np.asarray(qn_w, np.float32).reshape(1, D), (N_CORES, 1)),
        "kn_w": np.tile(np.asarray(kn_w, np.float32).reshape(1, D), (N_CORES, 1)),
        "id32": np.tile(np.eye(128, dtype=np.float32), (N_CORES, 1)),
        "id16": np.tile(np.eye(128, dtype=np.float16), (N_CORES, 1)),
    }

    wq_slices = [_f16(Wq[h * G * D:(h + 1) * G * D, :].T) for h in range(HK)]
    wkv_slices = [
        np.concatenate([_f16(Wk[h * D:(h + 1) * D, :].T),
                        _f16(Wv[h * D:(h + 1) * D, :].T)], axis=1)
        for h in range(HK)
    ]
    wo_slices = [_f16(Wo[:, h * G * D:(h + 1) * G * D].T) for h in range(HK)]

    def fill_core(c):
        h, gr = c % HK, c // HK
        bsl = slice(gr * BL, (gr + 1) * BL)
        g["x"][c * BL:(c + 1) * BL] = x[bsl].astype(np.float16)
        g["cos_t"][c * BL:(c + 1) * BL] = cos_t[bsl]
        g["sin_t"][c * BL:(c + 1) * BL] = sin_t[bsl]
        g["k_cache"][c * BL:(c + 1) * BL] = k_cache[bsl, h].astype(np.float16)
        g["v_cache"][c * BL:(c + 1) * BL] = _bf16(
            np.ascontiguousarray(v_cache[bsl, h])).reshape(BL, S_PAST, D)
        g["WqT"][c * DIM:(c + 1) * DIM] = wq_slices[h]
        g["WkvT"][c * DIM:(c + 1) * DIM] = wkv_slices[h]
        g["WoT"][c * G * D:(c + 1) * G * D] = wo_slices[h]

    with ThreadPoolExecutor(N_CORES) as ex:
        list(ex.map(fill_core, range(N_CORES)))
    return g


def _run_device(g):
    """One device call: upload global arrays, execute, download + merge y."""
    sharded, in_names, zero_shapes = _get_runner()
    args = [g[name] for name in in_names]
    zeros = [np.zeros((N_CORES * s[0],) + s[1:], dt) for s, dt in zero_shapes]
    out = sharded(*args, *zeros)
    yt = np.asarray(out[0]).reshape(N_CORES, DIM, BL)
    y = np.empty((B, DIM), np.float32)
    for gr in range(NB):
        acc = yt[gr * HK]
        for h in range(1, HK):
            acc = acc + yt[gr * HK + h]
        y[gr * BL:(gr + 1) * BL] = acc.T
    return y


def kernel(x, pos, k_cache, v_cache, Wq, Wk, Wv, Wo, qn_w, kn_w):
    _get_runner()
    g = _host_prep(x, pos, k_cache, v_cache, Wq, Wk, Wv, Wo, qn_w, kn_w)
    y = _run_device(g)
    return y.reshape(B, 1, DIM)


# revision 16
# speedup vs baseline: 3.4192x; 1.3112x over previous
"""GQA decode attention (B=64, T=1, HQ=16, HK=4, D=128, S_PAST=4095) on 8 TRN2 cores.

Under axon, wall time is dominated by the host->device tunnel (~60 MB/s), so the
design minimizes uploaded bytes and per-call host overhead:

  * Sharding: core c = (kv-head h = c%4, batch-half g = c//4). Each core gets
    its KV slice exactly once (no duplication) and only the weight slices for
    its 4 q-heads / 1 kv-head (~5 MB vs 42 MB full copy). Each core returns a
    partial y^T (its heads' o_proj contribution); host sums 4 partials per
    batch half.
  * dtypes over the wire: K cache f16 (upcast to f32 on device for the score
    matmuls), V cache bf16 (matches bf16 softmax weights in the PV matmul --
    bf16 is required there because exp(score-30) can exceed f16 max), weights
    f16, x f16. Total upload ~585 MB vs 1410 MB for the f32 data-parallel
    version.
  * Runner: the PJRT executable is built once and cached; global (stacked)
    input arrays are prebuilt in host prep, so a device call is transfer +
    execute + download only (no per-call np.concatenate / retrace).

Device pipeline per core: QKV projections (f16 matmul, f32 psum), per-head
RMSNorm + RoPE in f32, softmax attention over 4096 positions with a fixed
exp shift, o_proj for the core's 4 q-heads.
"""
import numpy as np
from contextlib import ExitStack

B, DIM = 64, 2048
HQ, HK, D = 16, 4, 128
G = HQ // HK          # 4 q-heads per kv head
S_PAST = 4095
S_FULL = S_PAST + 1
EPS = 1e-5
MIN_TS, MAX_TS = 1.0, 10000.0
N_CORES = 8
NB = 2                # batch halves
BL = B // NB          # 32 batches per core
NCH = S_FULL // 128   # 32 kv chunks of 128 positions
CG = 8                # chunks per exp group
EXP_SHIFT = 30.0      # fixed softmax shift; scores stay < ~60 so bf16/f32 hold


def _build_nc():
    import concourse.mybir as mybir
    from concourse import tile
    from concourse.bacc import Bacc

    f32 = mybir.dt.float32
    f16 = mybir.dt.float16
    bf16 = mybir.dt.bfloat16
    AF = mybir.ActivationFunctionType
    half1 = NCH // 2

    nc = Bacc()

    def reg_const(value):
        t = nc.alloc_sbuf_tensor(f"const-f32-{value}", [128, 1], f32)
        nc.gpsimd.memset(t.ap(), value)
        nc.const_aps.aps[(f32, value)] = t.ap()

    for _v in (EPS, 1.0 / D, -EXP_SHIFT):
        reg_const(float(_v))
    nc.all_engine_barrier()

    x_d = nc.declare_dram_parameter("x", [BL, DIM], f16, isOutput=False)
    cos_d = nc.declare_dram_parameter("cos_t", [BL, D // 2], f32, isOutput=False)
    sin_d = nc.declare_dram_parameter("sin_t", [BL, D // 2], f32, isOutput=False)
    i8 = mybir.dt.int8
    kc_d = nc.declare_dram_parameter("k_cache", [BL, S_PAST, D], f16, isOutput=False)
    vc_d = nc.declare_dram_parameter("v_cache", [BL, S_PAST, D], i8, isOutput=False)
    vsc_d = nc.declare_dram_parameter("v_scale", [BL, 128, NCH], f32, isOutput=False)
    wq_d = nc.declare_dram_parameter("WqT", [DIM, G * D], f16, isOutput=False)
    wkv_d = nc.declare_dram_parameter("WkvT", [DIM, 2 * D], f16, isOutput=False)
    wo_d = nc.declare_dram_parameter("WoT", [G * D, DIM], f16, isOutput=False)
    qnw_d = nc.declare_dram_parameter("qn_w", [1, D], f32, isOutput=False)
    knw_d = nc.declare_dram_parameter("kn_w", [1, D], f32, isOutput=False)
    id32_d = nc.declare_dram_parameter("id32", [128, 128], f32, isOutput=False)
    id16_d = nc.declare_dram_parameter("id16", [128, 128], f16, isOutput=False)
    yt_d = nc.declare_dram_parameter("yT", [DIM, BL], f16, isOutput=True)

    with tile.TileContext(nc) as tc, ExitStack() as ctx:
        cpool = ctx.enter_context(tc.tile_pool(name="const", bufs=1))
        spool = ctx.enter_context(tc.tile_pool(name="small", bufs=1))

        id32 = cpool.tile([128, 128], f32)
        nc.gpsimd.dma_start(id32[:], id32_d[:])
        id16 = cpool.tile([128, 128], f16)
        nc.gpsimd.dma_start(id16[:], id16_d[:])
        ones_row = cpool.tile([1, 128], f32)
        nc.gpsimd.memset(ones_row[:], 1.0)
        ones_col = cpool.tile([128, 1], bf16)
        nc.gpsimd.memset(ones_col[:], 1.0)
        zrow16 = cpool.tile([1, 128], f16)
        nc.gpsimd.memset(zrow16[:], 0.0)
        zrow8 = cpool.tile([1, 128], i8)
        nc.gpsimd.memset(zrow8[:], 0)
        qnw = cpool.tile([1, D], f32)
        nc.gpsimd.dma_start(qnw[:], qnw_d[:])
        knw = cpool.tile([1, D], f32)
        nc.gpsimd.dma_start(knw[:], knw_d[:])
        sin_t = cpool.tile([BL, D // 2], f32)
        nc.gpsimd.dma_start(sin_t[:], sin_d[:])
        cos_t = cpool.tile([BL, D // 2], f32)
        nc.gpsimd.dma_start(cos_t[:], cos_d[:])
        x_sb = cpool.tile([BL, DIM], f16)
        nc.gpsimd.dma_start(x_sb[:], x_d[:])

        # ---- phase 1: x.T chunks  [128, 16*BL], col kc*BL+b (f16) ----
        xT = cpool.tile([128, 16 * BL], f16)
        with tc.tile_pool(name="ph1ps", bufs=2, space="PSUM") as pps:
            # warm PE's clock on the ident DMA so later transposes carry a
            # single sync wait (walrus S3_LW allows only one)
            warm = pps.tile([128, 128], f16, tag="warm")
            nc.tensor.transpose(warm[:], id16[:], id16[:])
            for kc in range(16):
                ps = pps.tile([128, BL], f16)
                nc.tensor.transpose(ps[:], x_sb[:, kc * 128:(kc + 1) * 128],
                                    id16[0:BL, 0:BL])
                nc.vector.tensor_copy(xT[:, kc * BL:(kc + 1) * BL], ps[:])

        # ---- phase 2: projections -> q_bj [BL,512] f32, k/v_bj [BL,128] ----
        q_bj = cpool.tile([BL, G * D], f32)
        kv_bj = cpool.tile([BL, 2 * D], f32)
        with tc.tile_pool(name="wproj", bufs=2) as wpool, \
             tc.tile_pool(name="projps", bufs=2, space="PSUM") as ppool:
            pq = ppool.tile([BL, G * D], f32, tag="pq")
            pkv = ppool.tile([BL, 2 * D], f32, tag="pkv")
            for kc in range(16):
                wqt = wpool.tile([128, G * D], f16, tag="wq")
                nc.gpsimd.dma_start(wqt[:], wq_d[kc * 128:(kc + 1) * 128, :])
                wkvt = wpool.tile([128, 2 * D], f16, tag="wkv")
                nc.gpsimd.dma_start(wkvt[:], wkv_d[kc * 128:(kc + 1) * 128, :])
                lt = xT[:, kc * BL:(kc + 1) * BL]
                nc.tensor.matmul(pq[:], lhsT=lt, rhs=wqt[:],
                                 start=(kc == 0), stop=(kc == 15))
                nc.tensor.matmul(pkv[:], lhsT=lt, rhs=wkvt[:],
                                 start=(kc == 0), stop=(kc == 15))
            nc.vector.tensor_copy(q_bj[:], pq[:])
            nc.vector.tensor_copy(kv_bj[:], pkv[:])

        # ---- phase 3: rope tables + rmsnorm + rope (f32) ----
        q_rot = cpool.tile([BL, G * D], f32)
        k_rot = cpool.tile([BL, D], f32)
        with tc.tile_pool(name="ph3ps", bufs=4, space="PSUM") as pps, \
             tc.tile_pool(name="ph3sb", bufs=4) as tpool:
            tabs = {}
            for nm, w_sb in (("q", qnw), ("k", knw)):
                wb = pps.tile([BL, D], f32)
                nc.tensor.matmul(wb[:], lhsT=ones_row[:, 0:BL], rhs=w_sb[:],
                                 start=True, stop=True)
                cosA = spool.tile([BL, D // 2], f32, name=f"cosA_{nm}")
                nc.vector.tensor_mul(cosA[:], cos_t[:], wb[:, 0:D // 2])
                sinA = spool.tile([BL, D // 2], f32, name=f"sinA_{nm}")
                nc.vector.tensor_mul(sinA[:], sin_t[:], wb[:, D // 2:D])
                cosB = spool.tile([BL, D // 2], f32, name=f"cosB_{nm}")
                nc.vector.tensor_mul(cosB[:], cos_t[:], wb[:, D // 2:D])
                sinB = spool.tile([BL, D // 2], f32, name=f"sinB_{nm}")
                nc.vector.tensor_mul(sinB[:], sin_t[:], wb[:, 0:D // 2])
                tabs[nm] = (cosA, sinA, cosB, sinB)

            for src, dst, nheads, nm in ((q_bj, q_rot, G, "q"),
                                         (kv_bj, k_rot, 1, "k")):
                cosA, sinA, cosB, sinB = tabs[nm]
                for hh in range(nheads):
                    base = hh * D
                    hsl = src[:, base:base + D]
                    sq = tpool.tile([BL, D], f32, tag="sq")
                    nc.vector.tensor_mul(sq[:], hsl, hsl)
                    ssum = tpool.tile([BL, 1], f32, tag="ssum")
                    nc.vector.tensor_reduce(ssum[:], sq[:], axis=mybir.AxisListType.X,
                                            op=mybir.AluOpType.add)
                    sstd = tpool.tile([BL, 1], f32, tag="sstd")
                    nc.scalar.activation(sstd[:], ssum[:], AF.Sqrt, scale=1.0 / D, bias=EPS)
                    rstd = tpool.tile([BL, 1], f32, tag="rstd")
                    nc.vector.reciprocal(rstd[:], sstd[:])
                    qn = tpool.tile([BL, D], f32, tag="qn")
                    nc.vector.tensor_scalar_mul(qn[:], hsl, rstd[:])
                    h1, h2 = qn[:, 0:D // 2], qn[:, D // 2:D]
                    t1 = tpool.tile([BL, D // 2], f32, tag="t1")
                    nc.vector.tensor_mul(t1[:], h1, cosA[:])
                    t2 = tpool.tile([BL, D // 2], f32, tag="t2")
                    nc.vector.tensor_mul(t2[:], h2, sinA[:])
                    nc.vector.tensor_sub(dst[:, base:base + D // 2], t1[:], t2[:])
                    t3 = tpool.tile([BL, D // 2], f32, tag="t3")
                    nc.vector.tensor_mul(t3[:], h2, cosB[:])
                    t4 = tpool.tile([BL, D // 2], f32, tag="t4")
                    nc.vector.tensor_mul(t4[:], h1, sinB[:])
                    nc.vector.tensor_add(dst[:, base + D // 2:base + D], t3[:], t4[:])

        # ---- phase 4: q.T [128, G*BL] f32 ; k_new.T [128, BL] f32 ;
        #               v_bj as bf16 for the new-token PV matmul ----
        qT = cpool.tile([128, G * BL], f32)
        kTn = cpool.tile([128, BL], f32)
        v_bj16 = cpool.tile([BL, D], bf16)
        nc.vector.tensor_copy(v_bj16[:], kv_bj[:, D:2 * D])
        with tc.tile_pool(name="ph4ps", bufs=2, space="PSUM") as pps:
            for j in range(G):
                ps = pps.tile([128, BL], f32)
                nc.tensor.transpose(ps[:], q_rot[:, j * D:(j + 1) * D],
                                    id32[0:BL, 0:BL])
                nc.vector.tensor_copy(qT[:, j * BL:(j + 1) * BL], ps[:])
            ps = pps.tile([128, BL], f32)
            nc.tensor.transpose(ps[:], k_rot[:], id32[0:BL, 0:BL])
            nc.vector.tensor_copy(kTn[:], ps[:])

        # ---- phase 5: attention ----
        # psum cols: b*G + j  (j = local q-head)
        attn_n = cpool.tile([128, BL * G], f16)
        recip = spool.tile([1, BL * G], f32)
        with tc.tile_pool(name="kv", bufs=4) as kvpool, \
             tc.tile_pool(name="kts", bufs=4) as ktspool, \
             tc.tile_pool(name="exps", bufs=3) as expool, \
             tc.tile_pool(name="attnps", bufs=1, space="PSUM") as apspool, \
             tc.tile_pool(name="ktps", bufs=2, space="PSUM") as ktppool, \
             tc.tile_pool(name="scps", bufs=2, space="PSUM") as scppool:
            attn_ps = apspool.tile([128, BL * G], f32, tag="attn")
            den_ps = apspool.tile([1, BL * G], f32, tag="den")
            for b in range(BL):
                kt = kvpool.tile([128, S_FULL], f16, tag="k")
                vt = kvpool.tile([128, S_FULL], i8, tag="v")
                for tl, cache, zr in ((kt, kc_d, zrow16), (vt, vc_d, zrow8)):
                    nc.gpsimd.dma_start(
                        tl[:, 0:half1 * 128].rearrange("p (c d) -> p c d", d=128),
                        cache[b, 0:half1 * 128, :].rearrange(
                            "(c p) d -> p c d", p=128))
                    nc.gpsimd.dma_start(
                        tl[:, half1 * 128:(NCH - 1) * 128].rearrange(
                            "p (c d) -> p c d", d=128),
                        cache[b, half1 * 128:(NCH - 1) * 128, :].rearrange(
                            "(c p) d -> p c d", p=128))
                    nc.gpsimd.dma_start(
                        tl[0:127, (NCH - 1) * 128:S_FULL],
                        cache[b, (NCH - 1) * 128:S_PAST, :])
                    nc.gpsimd.dma_start(tl[127:128, (NCH - 1) * 128:S_FULL], zr[:])
                # partition 127 of the last chunk is garbage; consumers below
                # restrict that chunk to 127 rows. The appended (s=4095)
                # position is handled via kTn/v_bj16 rank-1 matmuls at the end.
                sct = ktspool.tile([128, NCH], f32, tag="sct")
                nc.gpsimd.dma_start(sct[:], vsc_d[b])
                vtb = kvpool.tile([128, S_FULL], bf16, tag="vb")
                for c in range(NCH):
                    nc.vector.tensor_scalar_mul(
                        vtb[:, c * 128:(c + 1) * 128],
                        vt[:, c * 128:(c + 1) * 128], sct[:, c:c + 1])
                rq = qT[:, b: b + (G - 1) * BL + 1: BL]
                for cg in range(NCH // CG):
                    sc = scppool.tile([128, CG * G], f32)
                    for j in range(CG):
                        c = cg * CG + j
                        ktp = ktppool.tile([128, 128], f16)
                        nc.tensor.transpose(ktp[:], kt[:, c * 128:(c + 1) * 128],
                                            id16[:])
                        kts = ktspool.tile([128, 128], f32)
                        nc.vector.tensor_copy(kts[:], ktp[:])
                        ns = 127 if c == NCH - 1 else 128
                        nc.tensor.matmul(sc[0:ns, j * G:(j + 1) * G],
                                         lhsT=kts[:, 0:ns],
                                         rhs=rq, start=True, stop=True)
                    ex = expool.tile([128, CG * G], bf16)
                    if cg == NCH // CG - 1:
                        nc.scalar.activation(ex[:, 0:(CG - 1) * G],
                                             sc[:, 0:(CG - 1) * G],
                                             AF.Exp, bias=-EXP_SHIFT)
                        nc.scalar.activation(ex[0:127, (CG - 1) * G:CG * G],
                                             sc[0:127, (CG - 1) * G:CG * G],
                                             AF.Exp, bias=-EXP_SHIFT)
                    else:
                        nc.scalar.activation(ex[:], sc[:], AF.Exp, bias=-EXP_SHIFT)
                    for j in range(CG):
                        c = cg * CG + j
                        ns = 127 if c == NCH - 1 else 128
                        nc.tensor.matmul(attn_ps[:, b * G:(b + 1) * G],
                                         lhsT=vtb[0:ns, c * 128:(c + 1) * 128],
                                         rhs=ex[0:ns, j * G:(j + 1) * G],
                                         start=(c == 0), stop=False,
                                         skip_group_check=True)
                        nc.tensor.matmul(den_ps[:, b * G:(b + 1) * G],
                                         lhsT=ones_col[0:ns],
                                         rhs=ex[0:ns, j * G:(j + 1) * G],
                                         start=(c == 0), stop=False,
                                         skip_group_check=True)
                # appended position s = S_PAST (new token)
                scn = scppool.tile([1, G], f32, tag="scn", bufs=1)
                nc.tensor.matmul(scn[:], lhsT=kTn[:, b:b + 1],
                                 rhs=rq, start=True, stop=True)
                exn = expool.tile([1, G], bf16, tag="exn")
                nc.scalar.activation(exn[:], scn[:], AF.Exp, bias=-EXP_SHIFT)
                vnew = expool.tile([1, D], bf16, tag="vnew")
                nc.gpsimd.dma_start(vnew[:], v_bj16[b:b + 1, :])
                nc.tensor.matmul(attn_ps[:, b * G:(b + 1) * G],
                                 lhsT=vnew[:],
                                 rhs=exn[:], start=False, stop=True,
                                 skip_group_check=True)
                nc.tensor.matmul(den_ps[:, b * G:(b + 1) * G],
                                 lhsT=ones_col[0:1],
                                 rhs=exn[:], start=False, stop=True,
                                 skip_group_check=True)
            # normalize: attn_n = attn_ps * (1/den) broadcast down partitions
            nc.vector.reciprocal(recip[:], den_ps[:])
            with tc.tile_pool(name="bcps", bufs=1, space="PSUM") as bcpool:
                bc = bcpool.tile([128, BL * G], f32)
                nc.tensor.matmul(bc[:], lhsT=ones_row[:], rhs=recip[:],
                                 start=True, stop=True)
                bcs = spool.tile([128, BL * G], f32)
                nc.vector.tensor_copy(bcs[:], bc[:])
                nc.vector.tensor_mul(attn_n[:], attn_ps[:], bcs[:])

        # ---- phase 6: o_proj  yT[n,b] = sum_{j,d} WoT[(j,d),n] attnT[(j,d),b] ----
        ysT = cpool.tile([128, 16 * BL], f16)
        with tc.tile_pool(name="wo", bufs=1) as wopool, \
             tc.tile_pool(name="yps", bufs=2, space="PSUM") as ypool:
            wts = []
            for j in range(G):
                wt = wopool.tile([128, DIM], f16, tag="wo", name=f"wo{j}", bufs=G)
                nc.gpsimd.dma_start(wt[:], wo_d[j * 128:(j + 1) * 128, :])
                wts.append(wt)
            for nh in range(16):
                yps = ypool.tile([128, BL], f32, tag="yps")
                for j in range(G):
                    rhs_a = attn_n[:, j: j + (BL - 1) * G + 1: G]
                    nc.tensor.matmul(yps[:],
                                     lhsT=wts[j][:, nh * 128:(nh + 1) * 128],
                                     rhs=rhs_a, start=(j == 0), stop=(j == G - 1))
                nc.vector.tensor_copy(ysT[:, nh * BL:(nh + 1) * BL], yps[:])
        nc.gpsimd.dma_start(
            yt_d[:].rearrange("(n p) b -> p n b", p=128),
            ysT[:].rearrange("p (n b) -> p n b", b=BL))

    return nc


_NC = None


def _get_nc():
    global _NC
    if _NC is None:
        nc = _build_nc()
        if not nc.is_finalized():
            nc.finalize()
        _NC = nc
    return _NC


_RUNNER = None


def _get_runner():
    """Build (once) the jitted shard_map executable for the bass module."""
    global _RUNNER
    if _RUNNER is None:
        import jax
        import concourse.mybir as mybir
        from concourse import bass2jax
        from jax.experimental.shard_map import shard_map
        from jax.sharding import Mesh, PartitionSpec

        nc = _get_nc()
        bass2jax.install_neuronx_cc_hook()
        assert nc.dbg_addr is None
        partition_name = (nc.partition_id_tensor.name
                          if nc.partition_id_tensor else None)

        in_names, out_names, out_avals, zero_shapes = [], [], [], []
        for alloc in nc.m.functions[0].allocations:
            if not isinstance(alloc, mybir.MemoryLocationSet):
                continue
            name = alloc.memorylocations[0].name
            if alloc.kind == "ExternalInput":
                if name != partition_name:
                    in_names.append(name)
            elif alloc.kind == "ExternalOutput":
                out_names.append(name)
                shape = tuple(alloc.tensor_shape)
                dtype = mybir.dt.np(alloc.dtype)
                out_avals.append(jax.core.ShapedArray(shape, dtype))
                zero_shapes.append((shape, dtype))
        n_params = len(in_names)
        all_names = list(in_names) + out_names
        if partition_name is not None:
            all_names.append(partition_name)

        def _body(*args):
            operands = list(args)
            if partition_name is not None:
                operands.append(bass2jax.partition_id_tensor())
            outs = bass2jax._bass_exec_p.bind(
                *operands,
                out_avals=tuple(out_avals),
                in_names=tuple(all_names),
                out_names=tuple(out_names),
                lowering_input_output_aliases=(),
                sim_require_finite=True,
                sim_require_nnan=True,
                nc=nc,
            )
            return tuple(outs)

        devices = jax.devices()[:N_CORES]
        mesh = Mesh(np.asarray(devices), ("core",))
        n_outs = len(out_names)
        sharded = jax.jit(
            shard_map(
                _body, mesh=mesh,
                in_specs=(PartitionSpec("core"),) * (n_params + n_outs),
                out_specs=(PartitionSpec("core"),) * n_outs,
                check_rep=False,
            ),
            donate_argnums=tuple(range(n_params, n_params + n_outs)),
            keep_unused=True,
        )
        _RUNNER = (sharded, in_names, zero_shapes)
    return _RUNNER


def _f16(a):
    return np.asarray(a, np.float32).astype(np.float16)


def _bf16(a32):
    """Round-to-nearest-even f32 -> bf16 via integer ops (fast, vectorized)."""
    import ml_dtypes
    u = np.ascontiguousarray(a32, np.float32).view(np.uint32)
    r = ((u + 0x7FFF + ((u >> 16) & 1)) >> 16).astype(np.uint16)
    return r.view(ml_dtypes.bfloat16)


def _host_prep(x, pos, k_cache, v_cache, Wq, Wk, Wv, Wo, qn_w, kn_w):
    """Build the global (core-stacked) input arrays the runner consumes."""
    from concurrent.futures import ThreadPoolExecutor

    x = np.asarray(x, np.float32).reshape(B, DIM)
    pos_f = np.asarray(pos, np.float64).reshape(B)
    half = D // 2
    fraction = 2.0 * np.arange(half, dtype=np.float64) / D
    inv_freq = 1.0 / (MIN_TS * (MAX_TS / MIN_TS) ** fraction)
    freqs = pos_f[:, None] * inv_freq[None, :]
    cos_t = np.cos(freqs).astype(np.float32)
    sin_t = np.sin(freqs).astype(np.float32)
    k_cache = np.asarray(k_cache, np.float32)
    v_cache = np.asarray(v_cache, np.float32)
    Wq = np.asarray(Wq, np.float32)
    Wk = np.asarray(Wk, np.float32)
    Wv = np.asarray(Wv, np.float32)
    Wo = np.asarray(Wo, np.float32)

    import ml_dtypes
    g = {
        "x": np.empty((N_CORES * BL, DIM), np.float16),
        "cos_t": np.empty((N_CORES * BL, half), np.float32),
        "sin_t": np.empty((N_CORES * BL, half), np.float32),
        "k_cache": np.empty((N_CORES * BL, S_PAST, D), np.float16),
        "v_cache": np.empty((N_CORES * BL, S_PAST, D), np.int8),
        "v_scale": np.empty((N_CORES * BL, 128, NCH), np.float32),
        "WqT": np.empty((N_CORES * DIM, G * D), np.float16),
        "WkvT": np.empty((N_CORES * DIM, 2 * D), np.float16),
        "WoT": np.empty((N_CORES * G * D, DIM), np.float16),
        "qn_w": np.tile(np.asarray(qn_w, np.float32).reshape(1, D), (N_CORES, 1)),
        "kn_w": np.tile(np.asarray(kn_w, np.float32).reshape(1, D), (N_CORES, 1)),
        "id32": np.tile(np.eye(128, dtype=np.float32), (N_CORES, 1)),
        "id16": np.tile(np.eye(128, dtype=np.float16), (N_CORES, 1)),
    }

    wq_slices = [_f16(Wq[h * G * D:(h + 1) * G * D, :].T) for h in range(HK)]
    wkv_slices = [
        np.concatenate([_f16(Wk[h * D:(h + 1) * D, :].T),
                        _f16(Wv[h * D:(h + 1) * D, :].T)], axis=1)
        for h in range(HK)
    ]
    wo_slices = [_f16(Wo[:, h * G * D:(h + 1) * G * D].T) for h in range(HK)]

    def fill_core(c):
        h, gr = c % HK, c // HK
        bsl = slice(gr * BL, (gr + 1) * BL)
        g["x"][c * BL:(c + 1) * BL] = x[bsl].astype(np.float16)
        g["cos_t"][c * BL:(c + 1) * BL] = cos_t[bsl]
        g["sin_t"][c * BL:(c + 1) * BL] = sin_t[bsl]
        g["k_cache"][c * BL:(c + 1) * BL] = k_cache[bsl, h].astype(np.float16)
        v = v_cache[bsl, h]                                   # [BL, S_PAST, D]
        sc = np.maximum(np.abs(v).max(axis=-1) / 127.0, 1e-20)
        g["v_cache"][c * BL:(c + 1) * BL] = np.rint(
            v / sc[..., None]).astype(np.int8)
        scp = np.zeros((BL, S_FULL), np.float32)
        scp[:, :S_PAST] = sc
        g["v_scale"][c * BL:(c + 1) * BL] = scp.reshape(
            BL, NCH, 128).transpose(0, 2, 1)
        g["WqT"][c * DIM:(c + 1) * DIM] = wq_slices[h]
        g["WkvT"][c * DIM:(c + 1) * DIM] = wkv_slices[h]
        g["WoT"][c * G * D:(c + 1) * G * D] = wo_slices[h]

    with ThreadPoolExecutor(N_CORES) as ex:
        list(ex.map(fill_core, range(N_CORES)))
    return g


def _run_device(g):
    """One device call: upload global arrays, execute, download + merge y."""
    sharded, in_names, zero_shapes = _get_runner()
    args = [g[name] for name in in_names]
    zeros = [np.zeros((N_CORES * s[0],) + s[1:], dt) for s, dt in zero_shapes]
    out = sharded(*args, *zeros)
    yt = np.asarray(out[0]).astype(np.float32).reshape(N_CORES, DIM, BL)
    y = np.empty((B, DIM), np.float32)
    for gr in range(NB):
        acc = yt[gr * HK]
        for h in range(1, HK):
            acc = acc + yt[gr * HK + h]
        y[gr * BL:(gr + 1) * BL] = acc.T
    return y


def kernel(x, pos, k_cache, v_cache, Wq, Wk, Wv, Wo, qn_w, kn_w):
    _get_runner()
    g = _host_prep(x, pos, k_cache, v_cache, Wq, Wk, Wv, Wo, qn_w, kn_w)
    y = _run_device(g)
    return y.reshape(B, 1, DIM)
